# revision 83
# baseline (speedup 1.0000x reference)
"""Causal multi-head attention block (QKV proj -> causal MHA -> out proj) on 8 Trainium2
cores.

Sharding: core = b*2 + hh handles batch b (of 4) and head-half hh (8 of 16 heads),
computing attention for its heads over the full sequence, then a partial output
projection over its 512 y-channels for all 2048 tokens. A pairwise ReduceScatter
([0,1],[2,3],...) sums the two partials of each batch and leaves each core with its
token-half of the final output.

Layout/precision: the host pre-transposes x to [C, T] and pre-lays-out all weights as
exact SBUF images in bf16, so the device does no transposes at all. All matmul operands
are bf16 (PSUM accumulation stays fp32); softmax, normalization, and the final output
are fp32. Causal masking multiplies the bf16 probs by a 0/1 triangle (DVE fast mode)
instead of adding -inf to scores.

Schedule: a single software-pipelined stream. For each query tile qt, the attention
inner loop (scores -> exp -> mask-mult -> attnV, per 128-token key block, double-
buffered through PSUM) is interleaved with the QKV projections of tile qt+1 and the
output projections of tile qt-1, keeping the PE busy while the Activation engine
(exp, the co-critical resource) drains.
"""

import numpy as np
import ml_dtypes

import concourse.bass as bass
import concourse.tile as tile
from concourse import bacc, library_config, mybir
from concourse.bass_utils import run_bass_kernel_spmd

F32 = mybir.dt.float32
BF16 = mybir.dt.bfloat16
AF = mybir.ActivationFunctionType

B, T, C, H = 4, 2048, 1024, 16
D = C // H          # 64
NHL = H // 2        # 8 local heads per core
NHP = NHL // 2      # 4 local head pairs
FL = NHL * D        # 512 local features
NCC = C // 128      # 8 contraction chunks over C
NTB = T // 128      # 16 token blocks
NTT = T // 512      # 4 token tiles / qtiles
VW = NHL * 65       # v_sb row stride per token block (8 heads x (64 d + 1 ones))


def build():
    nc = bacc.Bacc("TRN2", target_bir_lowering=False, num_devices=8)

    xtd = nc.dram_tensor("xtd", [128, NCC * T], BF16, kind="ExternalInput")
    wq_d = nc.dram_tensor("wq", [128, NCC * FL], BF16, kind="ExternalInput")
    wk_d = nc.dram_tensor("wk", [128, NCC * FL], BF16, kind="ExternalInput")
    wv_d = nc.dram_tensor("wv", [128, NCC * FL], BF16, kind="ExternalInput")
    wo_d = nc.dram_tensor("wo", [128, NHP * C], BF16, kind="ExternalInput")
    bq_d = nc.dram_tensor("bq", [128, NHP], F32, kind="ExternalInput")
    bk_d = nc.dram_tensor("bk", [128, NHP], F32, kind="ExternalInput")
    bvb_d = nc.dram_tensor("bvb", [128, FL], F32, kind="ExternalInput")
    bob_d = nc.dram_tensor("bob", [128, C], F32, kind="ExternalInput")  # bo/2 broadcast
    tri_d = nc.dram_tensor("tri", [128, 128], BF16, kind="ExternalInput")
    ident_d = nc.dram_tensor("ident", [128, 128], BF16, kind="ExternalInput")
    zh = nc.dram_tensor("zh", [T // 2, C], BF16, kind="ExternalOutput")

    with tile.TileContext(nc) as tc:
        with (
            tc.tile_pool(name="res", bufs=1) as res,
            tc.tile_pool(name="dram", bufs=1, space="DRAM") as dram,
            tc.tile_pool(name="mm_ps", bufs=2, space="PSUM") as mm_ps,
            tc.tile_pool(name="s_ps", bufs=2, space="PSUM") as s_ps,
            tc.tile_pool(name="yu_ps", bufs=2, space="PSUM") as yu_ps,
            tc.tile_pool(name="atp", bufs=8) as atp,
            tc.tile_pool(name="nrm", bufs=6) as nrm,
            tc.tile_pool(name="zp", bufs=5) as zp,
        ):
            wq_sb = res.tile([128, NCC * FL], BF16)
            wk_sb = res.tile([128, NCC * FL], BF16)
            wv_sb = res.tile([128, NCC * FL], BF16)
            wo_sb = res.tile([128, NHP * C], BF16)
            xt = res.tile([128, NCC * T], BF16)
            qt_sb = res.tile([128, NHP * T], BF16)
            kt_sb = res.tile([128, NHP * T], BF16)
            v_sb = res.tile([128, NTB * VW], BF16)
            ysb = res.tile([128, NHP * T], BF16)
            bq_sb = res.tile([128, NHP], F32)
            bk_sb = res.tile([128, NHP], F32)
            bvb_sb = res.tile([128, FL], F32)
            bob_sb = res.tile([128, C], F32)
            tri_sb = res.tile([128, 128], BF16)
            ident_sb = res.tile([128, 128], BF16)
            zpart = dram.tile([T, C], BF16)
            zreds = [dram.tile([128, C], BF16, name=f"zred{i}") for i in range(8)]

            xt3 = xt[:].rearrange("p (c t) -> p c t", c=NCC)
            xtd3 = xtd.rearrange("p (c t) -> p c t", c=NCC)

            # boot order: wk/x-tile-0 quarters interleaved so the first K
            # matmuls start after ~1.5us, then wq before Q units, wv before V
            QB = NCC // 4 * FL
            for q in range(4):
                nc.sync.dma_start(wk_sb[:, q * QB:(q + 1) * QB],
                                  wk_d[:, q * QB:(q + 1) * QB])
                nc.sync.dma_start(xt3[:, 2 * q:2 * q + 2, 0:512],
                                  xtd3[:, 2 * q:2 * q + 2, 0:512])
            HB = NCC // 2 * FL
            nc.sync.dma_start(bk_sb[:], bk_d[:, :])
            nc.sync.dma_start(wq_sb[:, 0:HB], wq_d[:, 0:HB])
            nc.sync.dma_start(wq_sb[:, HB:], wq_d[:, HB:])
            nc.sync.dma_start(bq_sb[:], bq_d[:, :])
            nc.sync.dma_start(wv_sb[:, 0:HB], wv_d[:, 0:HB])
            nc.sync.dma_start(wv_sb[:, HB:], wv_d[:, HB:])
            nc.sync.dma_start(bvb_sb[:], bvb_d[:, :])
            nc.sync.dma_start(tri_sb[:], tri_d[:, :])
            nc.sync.dma_start(ident_sb[:], ident_d[:, :])
            nc.sync.dma_start(bob_sb[:], bob_d[:, :])
            nc.sync.dma_start(wo_sb[:], wo_d[:, :])

            # warm the exp table (hides ACT_TABLE_LOAD under the first QKV tile)
            wt = nrm.tile([1, 1], F32, tag="warm")
            nc.gpsimd.memset(wt[:], 0.0)
            nc.scalar.activation(wt[:], wt[:], AF.Exp)

            # constant ones column of V (rowsum trick), written once
            v4 = v_sb[:].rearrange("p (t h c) -> p t h c", t=NTB, h=NHL)
            nc.gpsimd.memset(v4[:, :, :, D:D + 1], 1.0)

            # ---------------- work units ----------------

            def qk_unit(tt, w_sb, b_sb, dst, fb, nm):
                # two half-closures (4-cc matmul groups) for fine interleaving
                st = {}

                def p1():
                    st["ps"] = mm_ps.tile([128, 512], F32, tag="mm",
                                          name=f"{nm}{tt}_{fb}")
                    for cc in range(4):
                        nc.tensor.matmul(
                            st["ps"][:],
                            w_sb[:, cc * FL + fb * 128: cc * FL + (fb + 1) * 128],
                            xt[:, cc * T + tt * 512: cc * T + (tt + 1) * 512],
                            start=(cc == 0),
                            stop=False,
                        )

                def p2():
                    for cc in range(4, NCC):
                        nc.tensor.matmul(
                            st["ps"][:],
                            w_sb[:, cc * FL + fb * 128: cc * FL + (fb + 1) * 128],
                            xt[:, cc * T + tt * 512: cc * T + (tt + 1) * 512],
                            start=False,
                            stop=(cc == NCC - 1),
                        )
                    nc.vector.tensor_scalar_add(
                        dst[:, fb * T + tt * 512: fb * T + (tt + 1) * 512],
                        st["ps"][:],
                        b_sb[:, fb:fb + 1],
                    )
                return [p1, p2]

            def v_unit(tb):
                st = {}

                def p1():
                    st["ps"] = mm_ps.tile([128, 512], F32, tag="mm",
                                          name=f"v{tb}")
                    for cc in range(4):
                        nc.tensor.matmul(
                            st["ps"][:],
                            xt[:, cc * T + tb * 128: cc * T + (tb + 1) * 128],
                            wv_sb[:, cc * FL:(cc + 1) * FL],
                            start=(cc == 0),
                            stop=False,
                        )

                def p2():
                    for cc in range(4, NCC):
                        nc.tensor.matmul(
                            st["ps"][:],
                            xt[:, cc * T + tb * 128: cc * T + (tb + 1) * 128],
                            wv_sb[:, cc * FL:(cc + 1) * FL],
                            start=False,
                            stop=(cc == NCC - 1),
                        )
                    v3 = v_sb[:, tb * VW:(tb + 1) * VW].rearrange(
                        "p (h c) -> p h c", h=NHL)
                    nc.vector.tensor_add(
                        v3[:, :, 0:D],
                        st["ps"][:].rearrange("p (h d) -> p h d", h=NHL),
                        bvb_sb[:].rearrange("p (h d) -> p h d", h=NHL),
                    )
                return [p1, p2]

            def qkv_units(tt):
                us = []
                if tt > 0:
                    def xdma():
                        nc.sync.dma_start(
                            xt3[:, :, tt * 512:(tt + 1) * 512],
                            xtd3[:, :, tt * 512:(tt + 1) * 512],
                        )
                    us.append(xdma)
                for fb in range(NHP):
                    us += qk_unit(tt, wk_sb, bk_sb, kt_sb, fb, "k")
                    us += qk_unit(tt, wq_sb, bq_sb, qt_sb, fb, "q")
                    us += v_unit(4 * tt + fb)
                return us

            # zpart rows are chunk-major so each pairwise ReduceScatter chunk is a
            # contiguous 256-row block: chunk c = [tb c rows | tb 8+c rows].
            ZROW = {}
            for c in range(8):
                ZROW[c] = c * 256
                ZROW[8 + c] = c * 256 + 128

            def op_half(tb, ct):
                def emit():
                    zrow = ZROW[tb]
                    zps = mm_ps.tile([128, 512], F32, tag="mm",
                                     name=f"z{tb}_{ct}")
                    for cc in range(NHP):
                        nc.tensor.matmul(
                            zps[:],
                            ysb[:, cc * T + tb * 128: cc * T + (tb + 1) * 128],
                            wo_sb[:, cc * C + ct * 512: cc * C + (ct + 1) * 512],
                            start=(cc == 0),
                            stop=(cc == NHP - 1),
                        )
                    z_sb = zp.tile([128, 512], BF16, tag="z",
                                   name=f"zs{tb}_{ct}")
                    nc.vector.tensor_add(
                        z_sb[:], zps[:], bob_sb[:, ct * 512:(ct + 1) * 512])
                    nc.sync.dma_start(
                        zpart[zrow:zrow + 128, ct * 512:(ct + 1) * 512],
                        z_sb[:],
                    )
                return emit

            def op_units(qt):
                return [op_half(tb, ct)
                        for tb in range(4 * qt, 4 * qt + 4) for ct in range(2)]

            def att_closures(qt, hp):
                """Closures for one (qtile, head-pair): per key block kb emit
                scores+exp+mask, with attnV lagging one kb (PSUM double buffer)."""
                n_kb = 4 * (qt + 1)
                kb_order = list(range(n_kb))
                last_kb = kb_order[-1]
                st = {"at": {}}

                def start():
                    st["yus"] = [
                        yu_ps.tile([128, 4 * 65], F32, tag="yu",
                                   name=f"yu{qt}_{hp}_{i}")
                        for i in range(2)
                    ]

                def scores_kb(kb):
                    c = kb - 4 * qt
                    j0 = c * 128 if c > 0 else 0
                    ss = s_ps.tile([128, 1024], F32, tag="s",
                                   name=f"s{qt}_{hp}_{kb}")
                    for hi in range(2):
                        nc.tensor.matmul(
                            ss[:, hi * 512 + j0:(hi + 1) * 512],
                            kt_sb[hi * 64:(hi + 1) * 64,
                                  hp * T + kb * 128: hp * T + (kb + 1) * 128],
                            qt_sb[hi * 64:(hi + 1) * 64,
                                  hp * T + qt * 512 + j0: hp * T + (qt + 1) * 512],
                            tile_position=(hi * 64, 0),
                            start=True,
                            stop=True,
                        )
                    at = atp.tile([128, 1024], BF16, tag="at",
                                  name=f"at{qt}_{hp}_{kb}")
                    if j0 > 0:
                        ss3 = ss[:].rearrange("p (i x) -> p i x", i=2)
                        at3 = at[:].rearrange("p (i x) -> p i x", i=2)
                        nc.scalar.activation(
                            at3[:, :, j0:512], ss3[:, :, j0:512],
                            AF.Exp, scale=0.125)
                    else:
                        nc.scalar.activation(at[:], ss[:], AF.Exp, scale=0.125)
                    if 0 <= c <= 3:
                        for hi in range(2):
                            b0 = hi * 512 + c * 128
                            nc.vector.tensor_mul(
                                at[:, b0:b0 + 128], at[:, b0:b0 + 128], tri_sb[:])
                    st["at"][kb] = at

                def attnv_kb(kb):
                    c = kb - 4 * qt
                    at = st["at"].pop(kb)
                    for hi in range(2):
                        h = 2 * hp + hi
                        vsl = v_sb[:, kb * VW + h * 65: kb * VW + h * 65 + 65]
                        for qc in range(max(c, 0), 4):
                            # start=True zeroes the whole 2KB bank (the zero
                            # region), so ONLY the first matmul into each yu
                            # bank may set it; it pre-zeroes the sibling
                            # chunks' regions
                            nc.tensor.matmul(
                                st["yus"][hi][:, qc * 65:(qc + 1) * 65],
                                at[:, hi * 512 + qc * 128:
                                   hi * 512 + (qc + 1) * 128],
                                vsl,
                                start=(kb == 0 and qc == 0),
                                stop=(kb == 4 * qt + qc),
                            )

                def norm_a():
                    # rowsums are per-partition now: reciprocal + per-qc
                    # scalar multiply into a [q, d]-major bf16 staging tile
                    st["ystg"] = []
                    for hi in range(2):
                        yu3 = st["yus"][hi][:].rearrange(
                            "p (qc c) -> p qc c", qc=4)
                        rs = nrm.tile([128, 4], F32, tag="rs",
                                      name=f"rs{qt}_{hp}_{hi}")
                        nc.vector.reciprocal(rs[:], yu3[:, :, 64])
                        ystg = nrm.tile([128, 256], BF16, tag="ystg",
                                        name=f"ystg{qt}_{hp}_{hi}")
                        for qc in range(4):
                            nc.vector.tensor_scalar_mul(
                                ystg[:, qc * 64:(qc + 1) * 64],
                                yu3[:, qc, 0:64],
                                rs[:, qc:qc + 1],
                            )
                        st["ystg"].append(ystg)

                def norm_b():
                    # transpose [q, (qc,d)] pairs back to feature-major ysb
                    for hi in range(2):
                        ystg = st["ystg"][hi]
                        for tp in range(2):
                            tps = mm_ps.tile([128, 512], F32, tag="mm",
                                             name=f"yt{qt}_{hp}_{hi}_{tp}")
                            tpsb = tps[:].bitcast(BF16)
                            nc.tensor.transpose(
                                tpsb[:, 0:128],
                                ystg[:, tp * 128:(tp + 1) * 128],
                                ident_sb[:],
                            )
                            for half in range(2):
                                qc = 2 * tp + half
                                nc.vector.tensor_copy(
                                    ysb[hi * 64:(hi + 1) * 64,
                                        hp * T + qt * 512 + qc * 128:
                                        hp * T + qt * 512 + (qc + 1) * 128],
                                    tpsb[half * 64:(half + 1) * 64, 0:128],
                                )

                # attnV lags scores by 2 key blocks so the exp round-trip
                # (ACT busy + access latency + semaphores) is fully hidden
                cls = []

                def first():
                    start()
                    scores_kb(0)
                # process diagonal kbs early so the head-pair tail is plain
                # full blocks with no DVE mask hop on the exp->attnV chain;
                # PSUM accumulation order is free (start on first emitted,
                # stop on last)
                order = kb_order
                LAG = 6
                cls.append(first)
                for i in range(1, min(LAG, n_kb)):
                    cls.append(lambda kb=order[i]: scores_kb(kb))
                for i in range(LAG, n_kb):
                    def mid(kb=order[i], pkb=order[i - LAG]):
                        scores_kb(kb)
                        attnv_kb(pkb)
                    cls.append(mid)
                for i in range(max(0, n_kb - LAG), n_kb - 1):
                    cls.append(lambda kb=order[i]: attnv_kb(kb))

                def last():
                    attnv_kb(last_kb)
                    norm_a()
                cls.append(last)
                return cls, norm_b

            def rs_chunk(c):
                nc.gpsimd.collective_compute(
                    "ReduceScatter",
                    mybir.AluOpType.add,
                    replica_groups=[[0, 1], [2, 3], [4, 5], [6, 7]],
                    ins=[zpart[c * 256:(c + 1) * 256, :].opt()],
                    outs=[zreds[c].opt()],
                )
                nc.sync.dma_start(zh[c * 128:(c + 1) * 128, :], zreds[c][:])

            # ---------------- schedule ----------------
            # W0: QKV of tile 0, grouped K -> Q -> V to match weight DMA arrival
            for fb in range(NHP):
                for u in qk_unit(0, wk_sb, bk_sb, kt_sb, fb, "k"):
                    u()
            for fb in range(NHP):
                for u in qk_unit(0, wq_sb, bq_sb, qt_sb, fb, "q"):
                    u()
            for tb in range(4):
                for u in v_unit(tb):
                    u()
            # W1..W4: attention of qt interleaved with fillers. QKV(qt+1) fills
            # W1-W3; all output projections (no consumers until the collective)
            # fill W4, where the exp deficit is largest. A filler is pulled in
            # right after each head-pair's last step to cover the normalize
            # round-trip that frees its attnV accumulators.
            for qt in range(NTT):
                att, bonus = [], []
                # qt3-hp0 is pulled forward into W3 (qt2's window): W3 has ACT
                # slack while W4 is exp-bound, and its K/Q/V(tile3) deps are
                # spent early among W3's fillers
                hps = range(1, NHP) if qt == NTT - 1 else range(NHP)
                inserts = []
                for hp in hps:
                    cl, nb = att_closures(qt, hp)
                    att += cl
                    bonus += [0.0] * (len(cl) - 1) + [1.0]
                    inserts.append((len(att) + 2, nb))
                if qt == NTT - 2:
                    cl, nb = att_closures(NTT - 1, 0)
                    att += cl
                    bonus += [0.0] * (len(cl) - 1) + [1.0]
                    inserts.append((len(att) + 2, nb))
                # deferred y-transposes: 3 closures after their head-pair ends,
                # so the PE transpose never waits on the normalize multiplies
                for pos, nb in sorted(inserts, reverse=True):
                    if pos >= len(att):
                        att.append(nb)
                        bonus.append(0.0)
                    else:
                        att.insert(pos, nb)
                        bonus.insert(pos, 0.0)
                # W2/W3 get the first half of op0/op1 as reserves (independent
                # of the next window, unlike the QKV units whose copies gate
                # it); W4 gets the rest plus the RS chunks 0-3
                fillers = []
                if qt < NTT - 1:
                    fillers += qkv_units(qt + 1)
                    if qt == 1:
                        fillers += [op_half(0, 0), op_half(0, 1),
                                    op_half(1, 0), op_half(1, 1)]
                    elif qt == 2:
                        fillers += [op_half(4, 0), op_half(4, 1),
                                    op_half(5, 0), op_half(5, 1)]
                else:
                    for c in range(4):
                        if c >= 2:
                            fillers += [op_half(c, 0), op_half(c, 1)]
                        fillers += [op_half(8 + c, 0), op_half(8 + c, 1),
                                    (lambda c=c: rs_chunk(c))]
                    fillers += [op_half(6, 0), op_half(6, 1),
                                op_half(7, 0), op_half(7, 1)]
                # hold back a few fillers for the window tail, where the last
                # head-pair's exp/normalize chain drains with no att work left
                nrsv = min(8 if qt == NTT - 1 else 4, len(fillers))
                rsv = fillers[len(fillers) - nrsv:]
                spend = fillers[: len(fillers) - nrsv]
                r = len(spend) / len(att)
                acc, fi = 0.0, 0
                for a, bn in zip(att, bonus):
                    a()
                    acc += r + bn
                    while acc >= 1.0 and fi < len(spend):
                        spend[fi]()
                        fi += 1
                        acc -= 1.0
                while fi < len(spend):
                    spend[fi]()
                    fi += 1
                for u in rsv:
                    u()
            # W5: last out-proj tile, reduce-scatter chunks 4-7 as they complete
            ops3 = op_units(NTT - 1)
            for i, u in enumerate(ops3):
                u()
                if i % 2 == 1:
                    rs_chunk(4 + i // 2)

    nc.compile()
    return nc


_NC_CACHE = None


def _get_nc():
    global _NC_CACHE
    if _NC_CACHE is None:
        _NC_CACHE = build()
    return _NC_CACHE


def _in_maps(x, Wqkv, bqkv, Wo, bo):
    bf16 = ml_dtypes.bfloat16
    x = np.asarray(x, dtype=np.float32)
    Wqkv = np.asarray(Wqkv, dtype=np.float32)
    bqkv = np.asarray(bqkv, dtype=np.float32)
    Wo = np.asarray(Wo, dtype=np.float32)
    bo = np.asarray(bo, dtype=np.float32)

    i_ = np.arange(128)[:, None]
    j_ = np.arange(128)[None, :]
    tri = np.where(i_ > j_, 0.0, 1.0).astype(bf16)
    ident = np.eye(128).astype(bf16)

    def sbuf_img(w, dt=bf16):
        # [nch*128, f] -> [128, nch*f] SBUF image (partition-major chunks)
        nch = w.shape[0] // 128
        return np.ascontiguousarray(
            w.reshape(nch, 128, -1).transpose(1, 0, 2).reshape(128, -1)
        ).astype(dt)

    in_maps = []
    for core in range(8):
        b, hh = core // 2, core % 2
        sl = slice(hh * FL, (hh + 1) * FL)
        xt_img = sbuf_img(np.ascontiguousarray(x[b].T))
        bv_loc = bqkv[2 * C:3 * C][sl]
        in_maps.append({
            "xtd": xt_img,
            "wq": sbuf_img(np.ascontiguousarray(Wqkv[:, 0 * C:1 * C][:, sl])),
            "wk": sbuf_img(np.ascontiguousarray(Wqkv[:, 1 * C:2 * C][:, sl])),
            "wv": sbuf_img(np.ascontiguousarray(Wqkv[:, 2 * C:3 * C][:, sl])),
            "wo": sbuf_img(np.ascontiguousarray(Wo[sl, :])),
            "bq": np.ascontiguousarray(bqkv[0 * C:1 * C][sl].reshape(NHP, 128).T),
            "bk": np.ascontiguousarray(bqkv[1 * C:2 * C][sl].reshape(NHP, 128).T),
            "bvb": np.broadcast_to(bv_loc[None, :], (128, FL)).copy(),
            "bob": np.broadcast_to((bo * 0.5)[None, :], (128, C)).copy(),
            "tri": tri,
            "ident": ident,
        })
    return in_maps


def _assemble(res):
    out = np.empty((B, T, C), dtype=np.float32)
    for b in range(B):
        out[b, : T // 2] = np.asarray(res.results[2 * b]["zh"],
                                      dtype=np.float32)
        out[b, T // 2:] = np.asarray(res.results[2 * b + 1]["zh"],
                                     dtype=np.float32)
    return out


def kernel(x, Wqkv, bqkv, Wo, bo):
    in_maps = _in_maps(x, Wqkv, bqkv, Wo, bo)
    res = run_bass_kernel_spmd(_get_nc(), in_maps, core_ids=list(range(8)))
    return _assemble(res)


def run_traced(x, Wqkv, bqkv, Wo, bo, trace_cores=None):
    in_maps = _in_maps(x, Wqkv, bqkv, Wo, bo)
    res = run_bass_kernel_spmd(
        _get_nc(), in_maps, core_ids=list(range(8)), trace=True,
        trace_cores=trace_cores,
    )
    return res
